# revision 20
# baseline (speedup 1.0000x reference)
"""v7: et-major processing with h2-merged 32KB DMA rows for middle tiles.

Same math as v6; dw DRAM layout is [E, NCHUNK*K*CHUNK] (one 32KB row per
edge covering both tau chunks, parity-major lags within each chunk), so the
two middle edge-block transfers move 32KB packets (~26.3 GB/s/engine vs 23.2
at 16KB). Head/tail blocks split into column pieces for latency.
"""

import numpy as np

B, T, NVAR, K, E = 4, 2048, 128, 16, 512
TAU = T - 1
L = 1024
NC_COUNT = 8
W_XPAD = L + K         # 1040
ETILES = E // 128      # 4
CHUNK = 512
NCHUNK = L // CHUNK    # 2

_PROGRAM = None


def _build_program():
    import concourse.bass as bass
    import concourse.bacc as bacc
    import concourse.mybir as mybir
    import concourse.tile as tile

    f32 = mybir.dt.float32
    f32r = mybir.dt.float32r
    bf16 = mybir.dt.bfloat16
    nc = bacc.Bacc()

    ROW = NCHUNK * K * CHUNK  # 16384 cols per edge row (32KB bf16)
    HK = CHUNK * K // 2       # 4096 cols per parity quarter
    KH = K // 2

    xpad_d = nc.declare_dram_parameter("xpad", [NVAR, W_XPAD], bf16, isOutput=False)
    dw_d = nc.declare_dram_parameter("dw", [E, ROW], bf16, isOutput=False)
    ssend_d = nc.declare_dram_parameter("ssend", [NVAR, E], bf16, isOutput=False)
    wt_d = nc.declare_dram_parameter("wt", [NVAR, K * NVAR], bf16, isOutput=False)
    recv_d = nc.declare_dram_parameter("recvT", [128, ETILES * NVAR], bf16, isOutput=False)
    bo_d = nc.declare_dram_parameter("bias_ones", [1, NVAR + CHUNK], f32r, isOutput=False)
    y_d = nc.declare_dram_parameter("yT", [NVAR, L], f32, isOutput=True)

    with tile.TileContext(nc) as tc:
        with (
            tc.tile_pool(name="consts", bufs=1) as consts,
            tc.tile_pool(name="xgp", bufs=1) as xgp,
            tc.tile_pool(name="gpsum", bufs=4, space=bass.MemorySpace.PSUM) as gpsum,
            tc.tile_pool(name="dwp", bufs=3) as dwp,
            tc.tile_pool(name="prodp", bufs=2) as prodp,
            tc.tile_pool(name="opsum", bufs=2, space=bass.MemorySpace.PSUM) as opsum,
            tc.tile_pool(name="resp", bufs=2) as resp,
        ):
            xpad = consts.tile([NVAR, W_XPAD], bf16)
            nc.sync.dma_start(xpad[:], xpad_d[:])
            ssend = consts.tile([NVAR, E], bf16)
            nc.sync.dma_start(ssend[:], ssend_d[:])

            dwt_tiles = []
            for et in range(ETILES):
                dwt = dwp.tile([128, ROW], bf16, name="dwt", tag="dwt")
                dwt_tiles.append(dwt)
            wt = consts.tile([NVAR, K * NVAR], bf16)
            recvT = consts.tile([128, ETILES * NVAR], bf16)
            bias_ones = consts.tile([1, NVAR + CHUNK], f32r)

            def dw_cols(et, c0, c1):
                nc.sync.dma_start(
                    dwt_tiles[et][:, c0:c1], dw_d[et * 128:(et + 1) * 128, c0:c1]
                )

            # head block: parity quarters for chunk 0, then chunk-1 half;
            # middle blocks whole (32KB packets); tail block: chunk-0 half,
            # then parity quarters so the last multiply tracks the stream.
            dw_cols(0, 0, HK)
            dw_cols(0, HK, 2 * HK)
            dw_cols(0, 2 * HK, ROW)
            nc.sync.dma_start(recvT[:], recv_d[:])
            dw_cols(1, 0, ROW)
            nc.sync.dma_start(wt[:], wt_d[:])
            nc.sync.dma_start(bias_ones[:], bo_d[:])
            dw_cols(2, 0, ROW)
            dw_cols(3, 0, 2 * HK)
            dw_cols(3, 2 * HK, 3 * HK)
            dw_cols(3, 3 * HK, 3 * HK + HK // 2)
            dw_cols(3, 3 * HK + HK // 2, ROW)

            # gather sender rows (see v6)
            xg2 = []
            for et in range(ETILES):
                xt = xgp.tile([128, 2 * W_XPAD], bf16, name=f"xg{et}", tag=f"xg{et}")
                for j0 in range(0, W_XPAD, CHUNK):
                    jw = min(CHUNK, W_XPAD - j0)
                    gps = gpsum.tile([128, CHUNK], f32, name="gps", tag="gps")
                    nc.tensor.matmul(
                        gps[:, :jw],
                        ssend[:, et * 128:(et + 1) * 128],
                        xpad[:, j0:j0 + jw],
                        start=True, stop=True,
                    )
                    nc.scalar.copy(xt[:, j0:j0 + jw], gps[:, :jw])
                    if j0 == 0:
                        nc.scalar.copy(xt[:, W_XPAD:W_XPAD + jw - 1], gps[:, 1:jw])
                    else:
                        nc.scalar.copy(
                            xt[:, W_XPAD + j0 - 1:W_XPAD + j0 + jw - 1], gps[:, :jw]
                        )
                xg2.append(xt)

            ops_tiles = []
            for h2 in range(NCHUNK):
                o = opsum.tile([128, CHUNK], f32, name=f"ops{h2}", tag=f"ops{h2}")
                ops_tiles.append(o)

            def static_mm(h2, k, start=False):
                t0 = h2 * CHUNK
                nc.tensor.matmul(
                    ops_tiles[h2][:],
                    wt[:, k * NVAR:(k + 1) * NVAR],
                    xpad[:, t0 + k:t0 + k + CHUNK],
                    start=start, stop=False,
                )

            def bias_mm(h2):
                nc.tensor.matmul(
                    ops_tiles[h2][:],
                    bias_ones[:1, 0:NVAR],
                    bias_ones[:1, NVAR:NVAR + CHUNK],
                    start=False, stop=False,
                )

            # fills: chunk-1 statics+bias in groups 1/3/5 (first carries the
            # bank-1 start), chunk-0 in groups 2/4/6 (before group 6's stop)
            c0 = [(0, k) for k in range(K)] + [(0, None)]
            c1 = [(1, k) for k in range(K)] + [(1, None)]
            fills = {
                1: c1[0:6], 3: c1[6:12], 5: c1[12:17],
                2: c0[0:6], 4: c0[6:12], 6: c0[12:17],
            }

            def dyn_mm(pt, h2, et, plane, start):
                prow = pt.tensor.shape[-1]
                rhs = bass.AP(pt.tensor, plane * CHUNK,
                              [[prow, 128], [1, CHUNK]])
                nc.tensor.matmul(
                    ops_tiles[h2][:],
                    recvT[:, et * NVAR:(et + 1) * NVAR],
                    rhs,
                    start=start,
                    stop=(et == ETILES - 1 and plane == K - 1),
                )

            for g in range(2 * ETILES):
                et, h2 = divmod(g, 2)
                t0 = h2 * CHUNK
                dwt = dwt_tiles[et]
                drow = dwt.tensor.shape[-1]
                pt = prodp.tile([128, CHUNK * K], bf16, name="pt", tag="pt")
                prow = pt.tensor.shape[-1]
                xgt = xg2[et]
                xrow = xgt.tensor.shape[-1]
                for kind, karg in fills.get(g, []):
                    if karg is None:
                        bias_mm(kind)
                    else:
                        static_mm(kind, karg, start=(kind == 1 and karg == 0))
                cbase = h2 * 2 * HK
                if (g == 0) or (g == 2 * ETILES - 1):
                    if g == 0:
                        pieces = [(0, KH), (KH, K)]
                    else:
                        pieces = [(0, KH), (KH, KH + KH // 2), (KH + KH // 2, K)]
                    for p0, p1 in pieces:
                        par, m0 = divmod(p0, KH)
                        in1 = bass.AP(xgt.tensor, par * W_XPAD + t0 + 2 * m0,
                                      [[xrow, 128], [2, p1 - p0], [1, CHUNK]])
                        nc.vector.tensor_mul(
                            pt[:, p0 * CHUNK:p1 * CHUNK],
                            dwt[:, cbase + p0 * CHUNK:cbase + p1 * CHUNK],
                            in1,
                        )
                        for plane in range(p0, p1):
                            dyn_mm(pt, h2, et, plane,
                                   start=(g == 0 and plane == 0))
                else:
                    in1 = bass.AP(xgt.tensor, t0,
                                  [[xrow, 128], [W_XPAD, 2], [2, KH], [1, CHUNK]])
                    in0 = bass.AP(dwt.tensor, cbase,
                                  [[drow, 128], [HK, 2], [CHUNK, KH], [1, CHUNK]])
                    out4 = bass.AP(pt.tensor, 0,
                                   [[prow, 128], [HK, 2], [CHUNK, KH], [1, CHUNK]])
                    nc.vector.tensor_mul(out4, in0, in1)
                    for plane in range(K):
                        dyn_mm(pt, h2, et, plane, start=False)
                if et == ETILES - 1:
                    res = resp.tile([128, CHUNK], f32, name="res", tag="res")
                    if g == 2 * ETILES - 1:
                        nc.scalar.copy(res[:, 0:CHUNK // 2],
                                       ops_tiles[h2][:, 0:CHUNK // 2])
                        nc.vector.tensor_copy(res[:, CHUNK // 2:CHUNK],
                                              ops_tiles[h2][:, CHUNK // 2:CHUNK])
                    else:
                        nc.scalar.copy(res[:], ops_tiles[h2][:])
                    nc.scalar.dma_start(y_d[0:64, t0:t0 + CHUNK], res[0:64, :])
                    nc.sync.dma_start(y_d[64:128, t0:t0 + CHUNK], res[64:128, :])

    nc.compile()
    return nc


def _get_program():
    global _PROGRAM
    if _PROGRAM is None:
        _PROGRAM = _build_program()
    return _PROGRAM


def _host_prep(spikes, conv_weight, conv_bias, dyn_weights, edge_send, edge_recv):
    import ml_dtypes

    bf = ml_dtypes.bfloat16
    spikes = np.asarray(spikes, dtype=np.float32)
    conv_weight = np.asarray(conv_weight, dtype=np.float32)
    conv_bias = np.asarray(conv_bias, dtype=np.float32)
    dyn_weights = np.asarray(dyn_weights, dtype=np.float32)
    edge_send = np.asarray(edge_send, dtype=np.int64)
    edge_recv = np.asarray(edge_recv, dtype=np.int64)

    x = np.ascontiguousarray(spikes[..., 0].transpose(0, 2, 1))

    ssend = np.zeros((NVAR, E), bf)
    ssend[edge_send, np.arange(E)] = 1.0

    recvT = np.zeros((128, ETILES * NVAR), bf)
    for et in range(ETILES):
        rr = edge_recv[et * 128:(et + 1) * 128]
        recvT[np.arange(128), et * NVAR + rr] = 1.0

    w = conv_weight.copy()
    w[np.arange(NVAR), np.arange(NVAR), K - 1] = 0.0
    wt = np.ascontiguousarray(w.transpose(1, 2, 0)).reshape(NVAR, K * NVAR)
    wt = wt.astype(bf)

    bias_ones = np.concatenate(
        [conv_bias, np.ones(CHUNK, np.float32)]
    ).reshape(1, NVAR + CHUNK).astype(np.float32)

    kidx = np.concatenate([np.arange(0, K, 2), np.arange(1, K, 2)])

    in_maps = []
    for core in range(NC_COUNT):
        b, h = divmod(core, 2)
        tau0 = 0 if h == 0 else TAU - L
        xpad = np.zeros((NVAR, W_XPAD), np.float32)
        lo = tau0 - (K - 2)
        src_lo = max(lo, 0)
        xpad[:, src_lo - lo:W_XPAD - 1] = x[b, :, src_lo:tau0 + L + 1]
        a = dyn_weights[:, b, tau0:tau0 + L, :]          # [E, L, K]
        a = a.reshape(E, NCHUNK, CHUNK, K)               # [E, h2, tau, k]
        a = a.transpose(0, 1, 3, 2)                      # [E, h2, k, tau]
        a = a[:, :, kidx, :]                             # parity-major k
        dw = np.ascontiguousarray(a).reshape(E, NCHUNK * K * CHUNK)
        dw = dw.astype(bf)
        in_maps.append({
            "xpad": xpad.astype(bf),
            "dw": dw,
            "ssend": ssend,
            "wt": wt,
            "recvT": recvT,
            "bias_ones": bias_ones,
        })
    return in_maps


def _assemble(results):
    out = np.empty((B, TAU, NVAR, 1), np.float32)
    for core in range(NC_COUNT):
        b, h = divmod(core, 2)
        yT = results[core]["yT"]
        if h == 0:
            out[b, 0:L, :, 0] = yT.T
        else:
            out[b, L:TAU, :, 0] = yT[:, 1:L].T
    return out


def run_on_hw(in_maps, trace=False, **kwargs):
    from concourse.bass_utils import run_bass_kernel_spmd

    nc = _get_program()
    return run_bass_kernel_spmd(
        nc, in_maps, core_ids=list(range(NC_COUNT)), trace=trace, **kwargs
    )


def kernel(spikes, conv_weight, conv_bias, dyn_weights, edge_send, edge_recv):
    in_maps = _host_prep(
        spikes, conv_weight, conv_bias, dyn_weights, edge_send, edge_recv
    )
    res = run_on_hw(in_maps)
    return _assemble(res.results)


# revision 23
# speedup vs baseline: 1.3385x; 1.3385x over previous
"""Trainium2 Bass kernel for the spike-decoder GNN message-passing module.

Math (per batch b, output time tau in [0, T-2], variable v):
  out[b,tau,v] = bias[v]
               + sum_{i,k} w[v,i,k] * x[b,i,tau+k-(K-2)]          (static conv)
               + sum_{e: recv[e]=v} sum_k dw[e,b,tau,k] * x[b,send[e],tau+k-(K-2)]
with w = conv_weight masked at w[i,i,K-1] = 0, x = spikes[...,0] transposed to
[b, nvar, t], and out-of-range x treated as zero.

Sharding: 8 cores = (b in 0..3) x (time half h in 0..1). Each core computes a
1024-wide tau window ([0,1024) or [1023,2047) — one overlapping column keeps
shapes uniform for SPMD). dyn_weights streams in bf16 (16.8 MB/core) and is
the memory-bound stream; everything else is kept small and scheduled around
it (the dw tiles own the SP DMA queue; small constants slot between tiles,
outputs leave on the Scalar HWDGE queue).

On-core algorithm:
  - xg2[e,:] = x[send[e],:] gathered via one-hot matmul on PE (exact: x is
    0/1); stored in bf16 twice per tile: as-is and shifted one column, so
    every DVE window read starts 4B-aligned regardless of lag parity.
  - products P[e,(k,tau)] = dw_tile * sliding-window(xg2) on DVE. dw is laid
    out parity-major on host ([parity, k//2, tau]), so a tile is a single
    all-bf16 tensor_tensor (4D window AP, step 2) — stride-1 innermost + even
    bases keep the DVE in 2x packed mode. Head/tail tiles split per parity to
    match half-tile DMA granularity.
  - k-reduction + recv-scatter + transpose folded into PE: per plane, a
    matmul with stationary one-hot recv matrix and moving operand = strided
    columns of P, accumulating into PSUM[v, tau]
  - static conv: 16 matmuls with stationary wT_k and shifted xpad slices,
    issued as fills between the dyn-tile groups (PE is in-order; wt arrives
    mid-stream)
  - bias: rank-1 matmul (bias x ones)
All terms accumulate into one PSUM bank [v, 512] per tau chunk, copied out by
ScalarE. Output is [v, tau] per core; host transposes while assembling.
"""

import numpy as np

B, T, NVAR, K, E = 4, 2048, 128, 16, 512
TAU = T - 1            # 2047
L = 1024               # per-core tau window
NC_COUNT = 8
W_XPAD = L + K         # 1040 (1039 used; padded even)
ETILES = E // 128      # 4
CHUNK = 512            # tau chunk per PSUM bank
NCHUNK = L // CHUNK    # 2

_PROGRAM = None


def _build_program():
    import concourse.bass as bass
    import concourse.bacc as bacc
    import concourse.mybir as mybir
    import concourse.tile as tile

    f32 = mybir.dt.float32
    f32r = mybir.dt.float32r
    bf16 = mybir.dt.bfloat16
    # Bacc (not plain Bass): its compile pipeline runs generate_event_semaphores,
    # which splits multi-semaphore waits — a raw fp32 Matmult supports only one
    # sync-wait slot and walrus rejects more ("Too many sync wait commands").
    nc = bacc.Bacc()

    xpad_d = nc.declare_dram_parameter("xpad", [NVAR, W_XPAD], bf16, isOutput=False)
    dw_d = nc.declare_dram_parameter("dw", [NCHUNK * E, CHUNK * K], bf16, isOutput=False)
    ssend_d = nc.declare_dram_parameter("ssend", [NVAR, E], bf16, isOutput=False)
    wt_d = nc.declare_dram_parameter("wt", [NVAR, K * NVAR], bf16, isOutput=False)
    recv_d = nc.declare_dram_parameter("recvT", [128, ETILES * NVAR], bf16, isOutput=False)
    bo_d = nc.declare_dram_parameter("bias_ones", [1, NVAR + CHUNK], f32r, isOutput=False)
    y_d = nc.declare_dram_parameter("yT", [NVAR, L], f32, isOutput=True)

    with tile.TileContext(nc) as tc:
        with (
            tc.tile_pool(name="consts", bufs=1) as consts,
            tc.tile_pool(name="xgp", bufs=1) as xgp,
            tc.tile_pool(name="gpsum", bufs=4, space=bass.MemorySpace.PSUM) as gpsum,
            tc.tile_pool(name="dwp", bufs=6) as dwp,
            tc.tile_pool(name="prodp", bufs=2) as prodp,
            tc.tile_pool(name="opsum", bufs=2, space=bass.MemorySpace.PSUM) as opsum,
            tc.tile_pool(name="resp", bufs=2) as resp,
        ):
            NT = NCHUNK * ETILES  # 8 dw tiles
            HK = CHUNK * K // 2   # half-tile product columns (4096)
            KH = K // 2

            # SP/HWDGE issue order = completion order (per-engine FIFO). The
            # dw tiles own the queue; small constants are slotted so nothing
            # big delays the next tile the DVE is waiting for. Tiles 0 and 7
            # are split into half-DMAs so the first multiply starts earlier
            # and the tail half overlaps its matmuls.
            xpad = consts.tile([NVAR, W_XPAD], bf16)
            nc.sync.dma_start(xpad[:], xpad_d[:])
            ssend = consts.tile([NVAR, E], bf16)
            nc.sync.dma_start(ssend[:], ssend_d[:])

            def dw_dma(dwt, ti, halves):
                h2, et = divmod(ti, ETILES)
                r0 = h2 * E + et * 128
                if halves:
                    for half in range(2):
                        nc.sync.dma_start(
                            dwt[:, half * HK:(half + 1) * HK],
                            dw_d[r0:r0 + 128, half * HK:(half + 1) * HK],
                        )
                else:
                    nc.sync.dma_start(dwt[:], dw_d[r0:r0 + 128, :])

            dwt_tiles = []
            for ti in range(NT):
                dwt = dwp.tile([128, CHUNK * K], bf16, name="dwt", tag="dwt")
                dwt_tiles.append(dwt)
            wt = consts.tile([NVAR, K * NVAR], bf16)
            recvT = consts.tile([128, ETILES * NVAR], bf16)
            bias_ones = consts.tile([1, NVAR + CHUNK], f32r)

            dw_dma(dwt_tiles[0], 0, halves=True)
            dw_dma(dwt_tiles[1], 1, halves=False)
            nc.sync.dma_start(recvT[:], recv_d[:])
            dw_dma(dwt_tiles[2], 2, halves=False)
            nc.sync.dma_start(wt[:], wt_d[:])
            nc.sync.dma_start(bias_ones[:], bo_d[:])
            for ti in range(3, NT):
                dw_dma(dwt_tiles[ti], ti, halves=(ti >= NT - 2))

            # Gather sender rows into one combined tile per edge block:
            # xg2[et][p, j]        = xpad[send[et*128+p], j]      (even half)
            # xg2[et][p, WX + j]   = xpad[send[et*128+p], j+1]    (odd half)
            # bf16 is exact (x is 0/1). The odd half is shifted one column so
            # every lag window below starts at an even element offset.
            xg2 = []
            for et in range(ETILES):
                xt = xgp.tile([128, 2 * W_XPAD], bf16, name=f"xg{et}", tag=f"xg{et}")
                for j0 in range(0, W_XPAD, CHUNK):
                    jw = min(CHUNK, W_XPAD - j0)
                    gps = gpsum.tile([128, CHUNK], f32, name="gps", tag="gps")
                    nc.tensor.matmul(
                        gps[:, :jw],
                        ssend[:, et * 128:(et + 1) * 128],
                        xpad[:, j0:j0 + jw],
                        start=True, stop=True,
                    )
                    nc.scalar.copy(xt[:, j0:j0 + jw], gps[:, :jw])
                    if j0 == 0:
                        nc.scalar.copy(xt[:, W_XPAD:W_XPAD + jw - 1], gps[:, 1:jw])
                    else:
                        nc.scalar.copy(
                            xt[:, W_XPAD + j0 - 1:W_XPAD + j0 + jw - 1], gps[:, :jw]
                        )
                xg2.append(xt)

            ops_tiles = []
            for h2 in range(NCHUNK):
                o = opsum.tile([128, CHUNK], f32, name=f"ops{h2}", tag=f"ops{h2}")
                ops_tiles.append(o)

            def static_mm(h2, k, start=False):
                t0 = h2 * CHUNK
                nc.tensor.matmul(
                    ops_tiles[h2][:],
                    wt[:, k * NVAR:(k + 1) * NVAR],
                    xpad[:, t0 + k:t0 + k + CHUNK],
                    start=start, stop=False,
                )

            def bias_mm(h2):
                nc.tensor.matmul(
                    ops_tiles[h2][:],
                    bias_ones[:1, 0:NVAR],
                    bias_ones[:1, NVAR:NVAR + CHUNK],
                    start=False, stop=False,
                )

            # static conv + bias are issued as fills between dyn-tile groups
            # (PE is in-order and wt/xpad arrive while dw streams). Chunk-0
            # fills must precede tile 3's stop matmul; chunk-1 fills tile 7's.
            fills = {
                1: [(0, k) for k in range(6)],
                2: [(0, k) for k in range(6, 12)],
                3: [(0, k) for k in range(12, 16)] + [(0, None)],
                4: [(1, k) for k in range(6)],
                5: [(1, k) for k in range(6, 12)],
                6: [(1, k) for k in range(12, 16)] + [(1, None)],
            }

            def dyn_mm(pt, h2, et, plane, start):
                prow = pt.tensor.shape[-1]
                rhs = bass.AP(pt.tensor, plane * CHUNK,
                              [[prow, 128], [1, CHUNK]])
                nc.tensor.matmul(
                    ops_tiles[h2][:],
                    recvT[:, et * NVAR:(et + 1) * NVAR],
                    rhs,
                    start=start,
                    stop=(et == ETILES - 1 and plane == K - 1),
                )

            for ti in range(NT):
                h2, et = divmod(ti, ETILES)
                t0 = h2 * CHUNK
                dwt = dwt_tiles[ti]
                drow = dwt.tensor.shape[-1]
                pt = prodp.tile([128, CHUNK * K], bf16, name="pt", tag="pt")
                prow = pt.tensor.shape[-1]
                xgt = xg2[et]
                xrow = xgt.tensor.shape[-1]
                for kind, karg in fills.get(ti, []):
                    if karg is None:
                        bias_mm(kind)
                    else:
                        # the first matmul into PSUM bank 1 in PE order is
                        # the k=0 static fill at group 4 — it must carry
                        # start=True (bank 0 starts at tile 0's first dyn MM)
                        static_mm(kind, karg, start=(kind == 1 and karg == 0))
                # dw arrives parity-major: dwt[e, (par*KH + k//2)*CHUNK + tau].
                # Products keep that layout, so in0/out of each tensor_tensor
                # are contiguous and in1 reads lag-windows of xg2 with step 2
                # (parity selects the shifted half, stride W_XPAD) — every
                # innermost run starts 4B-aligned, keeping the bf16 DVE in 2x
                # packed mode.
                if ti in (0, NT - 2, NT - 1):
                    # head/tail tiles: one TT per parity half so compute
                    # starts/finishes at half-tile DMA granularity
                    for par in range(2):
                        in1 = bass.AP(xgt.tensor, par * W_XPAD + t0,
                                      [[xrow, 128], [2, KH], [1, CHUNK]])
                        nc.vector.tensor_mul(
                            pt[:, par * HK:(par + 1) * HK],
                            dwt[:, par * HK:(par + 1) * HK],
                            in1,
                        )
                        # k-reduction + recv scatter on PE (bf16, contiguous
                        # rhs): psum[v,tau] += sum_e recvT[e,v] * P[e, plane]
                        for m in range(KH):
                            plane = par * KH + m
                            dyn_mm(pt, h2, et, plane,
                                   start=(ti == 0 and plane == 0))
                else:
                    in1 = bass.AP(xgt.tensor, t0,
                                  [[xrow, 128], [W_XPAD, 2], [2, KH], [1, CHUNK]])
                    in0 = bass.AP(dwt.tensor, 0,
                                  [[drow, 128], [HK, 2], [CHUNK, KH], [1, CHUNK]])
                    out4 = bass.AP(pt.tensor, 0,
                                   [[prow, 128], [HK, 2], [CHUNK, KH], [1, CHUNK]])
                    nc.vector.tensor_mul(out4, in0, in1)
                    for plane in range(K):
                        dyn_mm(pt, h2, et, plane, start=False)
                if et == ETILES - 1:
                    res = resp.tile([128, CHUNK], f32, name="res", tag="res")
                    nc.scalar.copy(res[:], ops_tiles[h2][:])
                    # split by partition halves across BOTH HWDGE rings (keeps
                    # 2KB row packets, halves the tail drain time). The SP
                    # issue sits after every dw issue in program order, so it
                    # cannot delay the stream.
                    nc.scalar.dma_start(y_d[0:64, t0:t0 + CHUNK], res[0:64, :])
                    nc.sync.dma_start(y_d[64:128, t0:t0 + CHUNK], res[64:128, :])

    nc.compile()
    return nc


def _get_program():
    global _PROGRAM
    if _PROGRAM is None:
        _PROGRAM = _build_program()
    return _PROGRAM


def _host_prep(spikes, conv_weight, conv_bias, dyn_weights, edge_send, edge_recv):
    import ml_dtypes

    bf = ml_dtypes.bfloat16
    spikes = np.asarray(spikes, dtype=np.float32)
    conv_weight = np.asarray(conv_weight, dtype=np.float32)
    conv_bias = np.asarray(conv_bias, dtype=np.float32)
    dyn_weights = np.asarray(dyn_weights, dtype=np.float32)
    edge_send = np.asarray(edge_send, dtype=np.int64)
    edge_recv = np.asarray(edge_recv, dtype=np.int64)

    x = np.ascontiguousarray(spikes[..., 0].transpose(0, 2, 1))  # [B, NVAR, T]

    ssend = np.zeros((NVAR, E), bf)
    ssend[edge_send, np.arange(E)] = 1.0

    recvT = np.zeros((128, ETILES * NVAR), bf)
    for et in range(ETILES):
        rr = edge_recv[et * 128:(et + 1) * 128]
        recvT[np.arange(128), et * NVAR + rr] = 1.0

    w = conv_weight.copy()
    w[np.arange(NVAR), np.arange(NVAR), K - 1] = 0.0
    wt = np.ascontiguousarray(w.transpose(1, 2, 0)).reshape(NVAR, K * NVAR)
    wt = wt.astype(bf)

    bias_ones = np.concatenate(
        [conv_bias, np.ones(CHUNK, np.float32)]
    ).reshape(1, NVAR + CHUNK).astype(np.float32)

    # lag order: evens then odds (parity-major), matching the kernel's
    # contiguous tensor_tensor halves per tile
    kidx = np.concatenate([np.arange(0, K, 2), np.arange(1, K, 2)])

    in_maps = []
    for core in range(NC_COUNT):
        b, h = divmod(core, 2)
        tau0 = 0 if h == 0 else TAU - L  # 0 or 1023
        xpad = np.zeros((NVAR, W_XPAD), np.float32)
        lo = tau0 - (K - 2)  # first x column needed
        src_lo = max(lo, 0)
        xpad[:, src_lo - lo:W_XPAD - 1] = x[b, :, src_lo:tau0 + L + 1]
        a = dyn_weights[:, b, tau0:tau0 + L, :]          # [E, L, K]
        a = a.reshape(E, NCHUNK, CHUNK, K)               # [E, h2, tau, k]
        a = a.transpose(1, 0, 3, 2)                      # [h2, E, k, tau]
        a = a[:, :, kidx, :]                             # parity-major k
        dw = np.ascontiguousarray(a).reshape(NCHUNK * E, CHUNK * K)
        dw = dw.astype(bf)
        in_maps.append({
            "xpad": xpad.astype(bf),
            "dw": dw,
            "ssend": ssend,
            "wt": wt,
            "recvT": recvT,
            "bias_ones": bias_ones,
        })
    return in_maps


def _assemble(results):
    out = np.empty((B, TAU, NVAR, 1), np.float32)
    for core in range(NC_COUNT):
        b, h = divmod(core, 2)
        yT = results[core]["yT"]  # [NVAR, L]
        if h == 0:
            out[b, 0:L, :, 0] = yT.T
        else:
            out[b, L:TAU, :, 0] = yT[:, 1:L].T
    return out


def run_on_hw(in_maps, trace=False, **kwargs):
    from concourse.bass_utils import run_bass_kernel_spmd

    nc = _get_program()
    return run_bass_kernel_spmd(
        nc, in_maps, core_ids=list(range(NC_COUNT)), trace=trace, **kwargs
    )


def kernel(spikes, conv_weight, conv_bias, dyn_weights, edge_send, edge_recv):
    in_maps = _host_prep(
        spikes, conv_weight, conv_bias, dyn_weights, edge_send, edge_recv
    )
    res = run_on_hw(in_maps)
    return _assemble(res.results)


# revision 24
# speedup vs baseline: 1.3499x; 1.0085x over previous
"""Trainium2 Bass kernel for the spike-decoder GNN message-passing module.

Math (per batch b, output time tau in [0, T-2], variable v):
  out[b,tau,v] = bias[v]
               + sum_{i,k} w[v,i,k] * x[b,i,tau+k-(K-2)]          (static conv)
               + sum_{e: recv[e]=v} sum_k dw[e,b,tau,k] * x[b,send[e],tau+k-(K-2)]
with w = conv_weight masked at w[i,i,K-1] = 0, x = spikes[...,0] transposed to
[b, nvar, t], and out-of-range x treated as zero.

Sharding: 8 cores = (b in 0..3) x (time half h in 0..1). Each core computes a
1024-wide tau window ([0,1024) or [1023,2047) — one overlapping column keeps
shapes uniform for SPMD). dyn_weights streams in bf16 (16.8 MB/core) and is
the memory-bound stream; everything else is kept small and scheduled around
it (the dw tiles own the SP DMA queue; small constants slot between tiles,
outputs leave on the Scalar HWDGE queue).

On-core algorithm:
  - xg2[e,:] = x[send[e],:] gathered via one-hot matmul on PE (exact: x is
    0/1); stored in bf16 twice per tile: as-is and shifted one column, so
    every DVE window read starts 4B-aligned regardless of lag parity.
  - products P[e,(k,tau)] = dw_tile * sliding-window(xg2) on DVE. dw is laid
    out parity-major on host ([parity, k//2, tau]), so a tile is a single
    all-bf16 tensor_tensor (4D window AP, step 2) — stride-1 innermost + even
    bases keep the DVE in 2x packed mode. Head/tail tiles split per parity to
    match half-tile DMA granularity.
  - k-reduction + recv-scatter + transpose folded into PE: per plane, a
    matmul with stationary one-hot recv matrix and moving operand = strided
    columns of P, accumulating into PSUM[v, tau]
  - static conv: 16 matmuls with stationary wT_k and shifted xpad slices,
    issued as fills between the dyn-tile groups (PE is in-order; wt arrives
    mid-stream)
  - bias: rank-1 matmul (bias x ones)
All terms accumulate into one PSUM bank [v, 512] per tau chunk, copied out by
ScalarE. Output is [v, tau] per core; host transposes while assembling.
"""

import numpy as np

B, T, NVAR, K, E = 4, 2048, 128, 16, 512
TAU = T - 1            # 2047
L = 1024               # per-core tau window
NC_COUNT = 8
W_XPAD = L + K         # 1040 (1039 used; padded even)
ETILES = E // 128      # 4
CHUNK = 512            # tau chunk per PSUM bank
NCHUNK = L // CHUNK    # 2

_PROGRAM = None


def _build_program():
    import concourse.bass as bass
    import concourse.bacc as bacc
    import concourse.mybir as mybir
    import concourse.tile as tile

    f32 = mybir.dt.float32
    f32r = mybir.dt.float32r
    bf16 = mybir.dt.bfloat16
    # Bacc (not plain Bass): its compile pipeline runs generate_event_semaphores,
    # which splits multi-semaphore waits — a raw fp32 Matmult supports only one
    # sync-wait slot and walrus rejects more ("Too many sync wait commands").
    nc = bacc.Bacc()

    xpad_d = nc.declare_dram_parameter("xpad", [NVAR, W_XPAD], bf16, isOutput=False)
    dw_d = nc.declare_dram_parameter("dw", [NCHUNK * E, CHUNK * K], bf16, isOutput=False)
    ssend_d = nc.declare_dram_parameter("ssend", [NVAR, E], bf16, isOutput=False)
    wt_d = nc.declare_dram_parameter("wt", [NVAR, K * NVAR], bf16, isOutput=False)
    recv_d = nc.declare_dram_parameter("recvT", [128, ETILES * NVAR], bf16, isOutput=False)
    bo_d = nc.declare_dram_parameter("bias_ones", [1, NVAR + CHUNK], f32r, isOutput=False)
    y_d = nc.declare_dram_parameter("yT", [NVAR, L], f32, isOutput=True)

    with tile.TileContext(nc) as tc:
        with (
            tc.tile_pool(name="consts", bufs=1) as consts,
            tc.tile_pool(name="xgp", bufs=1) as xgp,
            tc.tile_pool(name="gpsum", bufs=4, space=bass.MemorySpace.PSUM) as gpsum,
            tc.tile_pool(name="dwp", bufs=6) as dwp,
            tc.tile_pool(name="prodp", bufs=2) as prodp,
            tc.tile_pool(name="opsum", bufs=2, space=bass.MemorySpace.PSUM) as opsum,
            tc.tile_pool(name="resp", bufs=2) as resp,
        ):
            NT = NCHUNK * ETILES  # 8 dw tiles
            HK = CHUNK * K // 2   # half-tile product columns (4096)
            KH = K // 2

            # SP/HWDGE issue order = completion order (per-engine FIFO). The
            # dw tiles own the queue; small constants are slotted so nothing
            # big delays the next tile the DVE is waiting for. Tiles 0 and 7
            # are split into half-DMAs so the first multiply starts earlier
            # and the tail half overlaps its matmuls.
            xpad = consts.tile([NVAR, W_XPAD], bf16)
            nc.sync.dma_start(xpad[:], xpad_d[:])
            ssend = consts.tile([NVAR, E], bf16)
            nc.sync.dma_start(ssend[:], ssend_d[:])

            def dw_dma(dwt, ti, halves):
                h2, et = divmod(ti, ETILES)
                r0 = h2 * E + et * 128
                if halves:
                    for half in range(2):
                        nc.sync.dma_start(
                            dwt[:, half * HK:(half + 1) * HK],
                            dw_d[r0:r0 + 128, half * HK:(half + 1) * HK],
                        )
                else:
                    nc.sync.dma_start(dwt[:], dw_d[r0:r0 + 128, :])

            dwt_tiles = []
            for ti in range(NT):
                dwt = dwp.tile([128, CHUNK * K], bf16, name="dwt", tag="dwt")
                dwt_tiles.append(dwt)
            wt = consts.tile([NVAR, K * NVAR], bf16)
            recvT = consts.tile([128, ETILES * NVAR], bf16)
            bias_ones = consts.tile([1, NVAR + CHUNK], f32r)

            dw_dma(dwt_tiles[0], 0, halves=True)
            dw_dma(dwt_tiles[1], 1, halves=False)
            nc.sync.dma_start(recvT[:], recv_d[:])
            dw_dma(dwt_tiles[2], 2, halves=False)
            nc.sync.dma_start(wt[:], wt_d[:])
            nc.sync.dma_start(bias_ones[:], bo_d[:])
            for ti in range(3, NT):
                dw_dma(dwt_tiles[ti], ti, halves=(ti == NT - 1))

            # Gather sender rows into one combined tile per edge block:
            # xg2[et][p, j]        = xpad[send[et*128+p], j]      (even half)
            # xg2[et][p, WX + j]   = xpad[send[et*128+p], j+1]    (odd half)
            # bf16 is exact (x is 0/1). The odd half is shifted one column so
            # every lag window below starts at an even element offset.
            xg2 = []
            for et in range(ETILES):
                xt = xgp.tile([128, 2 * W_XPAD], bf16, name=f"xg{et}", tag=f"xg{et}")
                for j0 in range(0, W_XPAD, CHUNK):
                    jw = min(CHUNK, W_XPAD - j0)
                    gps = gpsum.tile([128, CHUNK], f32, name="gps", tag="gps")
                    nc.tensor.matmul(
                        gps[:, :jw],
                        ssend[:, et * 128:(et + 1) * 128],
                        xpad[:, j0:j0 + jw],
                        start=True, stop=True,
                    )
                    nc.scalar.copy(xt[:, j0:j0 + jw], gps[:, :jw])
                    if j0 == 0:
                        nc.scalar.copy(xt[:, W_XPAD:W_XPAD + jw - 1], gps[:, 1:jw])
                    else:
                        nc.scalar.copy(
                            xt[:, W_XPAD + j0 - 1:W_XPAD + j0 + jw - 1], gps[:, :jw]
                        )
                xg2.append(xt)

            ops_tiles = []
            for h2 in range(NCHUNK):
                o = opsum.tile([128, CHUNK], f32, name=f"ops{h2}", tag=f"ops{h2}")
                ops_tiles.append(o)

            def static_mm(h2, k, start=False):
                t0 = h2 * CHUNK
                nc.tensor.matmul(
                    ops_tiles[h2][:],
                    wt[:, k * NVAR:(k + 1) * NVAR],
                    xpad[:, t0 + k:t0 + k + CHUNK],
                    start=start, stop=False,
                )

            def bias_mm(h2):
                nc.tensor.matmul(
                    ops_tiles[h2][:],
                    bias_ones[:1, 0:NVAR],
                    bias_ones[:1, NVAR:NVAR + CHUNK],
                    start=False, stop=False,
                )

            # static conv + bias are issued as fills between dyn-tile groups
            # (PE is in-order and wt/xpad arrive while dw streams). Chunk-0
            # fills must precede tile 3's stop matmul; chunk-1 fills tile 7's.
            fills = {
                1: [(0, k) for k in range(6)],
                2: [(0, k) for k in range(6, 12)],
                3: [(0, k) for k in range(12, 16)] + [(0, None)],
                4: [(1, k) for k in range(6)],
                5: [(1, k) for k in range(6, 12)],
                6: [(1, k) for k in range(12, 16)] + [(1, None)],
            }

            def dyn_mm(pt, h2, et, plane, start):
                prow = pt.tensor.shape[-1]
                rhs = bass.AP(pt.tensor, plane * CHUNK,
                              [[prow, 128], [1, CHUNK]])
                nc.tensor.matmul(
                    ops_tiles[h2][:],
                    recvT[:, et * NVAR:(et + 1) * NVAR],
                    rhs,
                    start=start,
                    stop=(et == ETILES - 1 and plane == K - 1),
                )

            for ti in range(NT):
                h2, et = divmod(ti, ETILES)
                t0 = h2 * CHUNK
                dwt = dwt_tiles[ti]
                drow = dwt.tensor.shape[-1]
                pt = prodp.tile([128, CHUNK * K], bf16, name="pt", tag="pt")
                prow = pt.tensor.shape[-1]
                xgt = xg2[et]
                xrow = xgt.tensor.shape[-1]
                for kind, karg in fills.get(ti, []):
                    if karg is None:
                        bias_mm(kind)
                    else:
                        # the first matmul into PSUM bank 1 in PE order is
                        # the k=0 static fill at group 4 — it must carry
                        # start=True (bank 0 starts at tile 0's first dyn MM)
                        static_mm(kind, karg, start=(kind == 1 and karg == 0))
                # dw arrives parity-major: dwt[e, (par*KH + k//2)*CHUNK + tau].
                # Products keep that layout, so in0/out of each tensor_tensor
                # are contiguous and in1 reads lag-windows of xg2 with step 2
                # (parity selects the shifted half, stride W_XPAD) — every
                # innermost run starts 4B-aligned, keeping the bf16 DVE in 2x
                # packed mode.
                if ti in (0, NT - 1):
                    # head/tail tiles: one TT per parity half so compute
                    # starts/finishes at half-tile DMA granularity
                    for par in range(2):
                        in1 = bass.AP(xgt.tensor, par * W_XPAD + t0,
                                      [[xrow, 128], [2, KH], [1, CHUNK]])
                        nc.vector.tensor_mul(
                            pt[:, par * HK:(par + 1) * HK],
                            dwt[:, par * HK:(par + 1) * HK],
                            in1,
                        )
                        # k-reduction + recv scatter on PE (bf16, contiguous
                        # rhs): psum[v,tau] += sum_e recvT[e,v] * P[e, plane]
                        for m in range(KH):
                            plane = par * KH + m
                            dyn_mm(pt, h2, et, plane,
                                   start=(ti == 0 and plane == 0))
                else:
                    in1 = bass.AP(xgt.tensor, t0,
                                  [[xrow, 128], [W_XPAD, 2], [2, KH], [1, CHUNK]])
                    in0 = bass.AP(dwt.tensor, 0,
                                  [[drow, 128], [HK, 2], [CHUNK, KH], [1, CHUNK]])
                    out4 = bass.AP(pt.tensor, 0,
                                   [[prow, 128], [HK, 2], [CHUNK, KH], [1, CHUNK]])
                    nc.vector.tensor_mul(out4, in0, in1)
                    for plane in range(K):
                        dyn_mm(pt, h2, et, plane, start=False)
                if et == ETILES - 1:
                    res = resp.tile([128, CHUNK], f32, name="res", tag="res")
                    nc.scalar.copy(res[:], ops_tiles[h2][:])
                    # split by partition halves across BOTH HWDGE rings (keeps
                    # 2KB row packets, halves the tail drain time). The SP
                    # issue sits after every dw issue in program order, so it
                    # cannot delay the stream.
                    nc.scalar.dma_start(y_d[0:64, t0:t0 + CHUNK], res[0:64, :])
                    nc.sync.dma_start(y_d[64:128, t0:t0 + CHUNK], res[64:128, :])

    nc.compile()
    return nc


def _get_program():
    global _PROGRAM
    if _PROGRAM is None:
        _PROGRAM = _build_program()
    return _PROGRAM


def _host_prep(spikes, conv_weight, conv_bias, dyn_weights, edge_send, edge_recv):
    import ml_dtypes

    bf = ml_dtypes.bfloat16
    spikes = np.asarray(spikes, dtype=np.float32)
    conv_weight = np.asarray(conv_weight, dtype=np.float32)
    conv_bias = np.asarray(conv_bias, dtype=np.float32)
    dyn_weights = np.asarray(dyn_weights, dtype=np.float32)
    edge_send = np.asarray(edge_send, dtype=np.int64)
    edge_recv = np.asarray(edge_recv, dtype=np.int64)

    x = np.ascontiguousarray(spikes[..., 0].transpose(0, 2, 1))  # [B, NVAR, T]

    ssend = np.zeros((NVAR, E), bf)
    ssend[edge_send, np.arange(E)] = 1.0

    recvT = np.zeros((128, ETILES * NVAR), bf)
    for et in range(ETILES):
        rr = edge_recv[et * 128:(et + 1) * 128]
        recvT[np.arange(128), et * NVAR + rr] = 1.0

    w = conv_weight.copy()
    w[np.arange(NVAR), np.arange(NVAR), K - 1] = 0.0
    wt = np.ascontiguousarray(w.transpose(1, 2, 0)).reshape(NVAR, K * NVAR)
    wt = wt.astype(bf)

    bias_ones = np.concatenate(
        [conv_bias, np.ones(CHUNK, np.float32)]
    ).reshape(1, NVAR + CHUNK).astype(np.float32)

    # lag order: evens then odds (parity-major), matching the kernel's
    # contiguous tensor_tensor halves per tile
    kidx = np.concatenate([np.arange(0, K, 2), np.arange(1, K, 2)])

    in_maps = []
    for core in range(NC_COUNT):
        b, h = divmod(core, 2)
        tau0 = 0 if h == 0 else TAU - L  # 0 or 1023
        xpad = np.zeros((NVAR, W_XPAD), np.float32)
        lo = tau0 - (K - 2)  # first x column needed
        src_lo = max(lo, 0)
        xpad[:, src_lo - lo:W_XPAD - 1] = x[b, :, src_lo:tau0 + L + 1]
        a = dyn_weights[:, b, tau0:tau0 + L, :]          # [E, L, K]
        a = a.reshape(E, NCHUNK, CHUNK, K)               # [E, h2, tau, k]
        a = a.transpose(1, 0, 3, 2)                      # [h2, E, k, tau]
        a = a[:, :, kidx, :]                             # parity-major k
        dw = np.ascontiguousarray(a).reshape(NCHUNK * E, CHUNK * K)
        dw = dw.astype(bf)
        in_maps.append({
            "xpad": xpad.astype(bf),
            "dw": dw,
            "ssend": ssend,
            "wt": wt,
            "recvT": recvT,
            "bias_ones": bias_ones,
        })
    return in_maps


def _assemble(results):
    out = np.empty((B, TAU, NVAR, 1), np.float32)
    for core in range(NC_COUNT):
        b, h = divmod(core, 2)
        yT = results[core]["yT"]  # [NVAR, L]
        if h == 0:
            out[b, 0:L, :, 0] = yT.T
        else:
            out[b, L:TAU, :, 0] = yT[:, 1:L].T
    return out


def run_on_hw(in_maps, trace=False, **kwargs):
    from concourse.bass_utils import run_bass_kernel_spmd

    nc = _get_program()
    return run_bass_kernel_spmd(
        nc, in_maps, core_ids=list(range(NC_COUNT)), trace=trace, **kwargs
    )


def kernel(spikes, conv_weight, conv_bias, dyn_weights, edge_send, edge_recv):
    in_maps = _host_prep(
        spikes, conv_weight, conv_bias, dyn_weights, edge_send, edge_recv
    )
    res = run_on_hw(in_maps)
    return _assemble(res.results)
